# revision 8
# baseline (speedup 1.0000x reference)
"""DeepSet-equivariant layer on 8 TRN2 NeuronCores.

Math (reference):
    y = x @ w1 + (colsum(x) @ w2) / n + bias        x: (n, 128)

Distribution: shard x and y along the set dimension n across the 8 cores;
w1/w2/bias replicated. Each core exchanges a 128-vector with one
remote_dma_broadcast (SBUF -> 7 peers' SBUF).

v2 schedule — the key idea vs v1: x@w1 does NOT depend on the pooled
term, so all matmul + PSUM-drain work runs inside the input-DMA shadow.
The column-sum is obtained for free by fusing accum_out into every
PSUM drain: drains produce colsum(lin) chunk sums, where lin = x@w1.
Since colsum(lin) = colsum(x) @ w1, the host folds w1^{-1} into w2
(w2x = solve(w1, w2)/n, fp64) and the device computes
t = global_colsum_lin @ w2x + bias. Verified: identical rel-err to the
colsum(x) path (2.86e-3), transmit-term error ~0.1%.

Per core:
  - 8 dummy matmuls on zeros at t=0 warm the PE HAM clock gate (~3.4us
    cold budget) so the real matmuls run at 2.4 GHz, not 1.2.
  - input xT streamed bf16 into resident SBUF: first chunks alternate
    Scalar/Sync HWDGE rings, then Sync; tapered tail so the last drain
    (which completes the local colsum) is short.
  - per 2048-col PSUM tile: matmuls (N=512), then drain PSUM->ob (bf16)
    with accum_out -> cs_parts column; drains split DVE/ACT to balance
    (DVE 0.96 GHz, ACT 1.2 GHz, both 1x from PSUM).
  - local colsum = reduce(cs_parts) -> gather slot 0; one
    remote_dma_broadcast sends it to slot 8+my_id on the 7 peers; after
    14 remote-sem incs the 16 slots are reduced, t = SL @ w2x + bias.
  - phase 3: per out chunk, in-place ob += t on DVE (bf16 SBUF 4x mode),
    then DMA out alternating Sync/Scalar rings.

Tile's single-core scheduling sim cannot model remote arrivals, so the
two protocol waits are emitted after the TileContext and spliced into
engine-queue position by direct BIR list surgery (as in v1).
"""

import numpy as np
import ml_dtypes

import concourse.bass as bass
import concourse.tile as tile
from concourse import bacc, mybir
from concourse.bass_utils import run_bass_kernel_spmd

N_CORES = 8
D = 128                 # d_in == d_out
N_ROWS = 200000         # full set size
R = 25088               # padded rows per core: 8 * 25088 = 200704 >= 200000
PS_N = 1024             # columns per PSUM tile (2 banks fp32, 4 bufs)
MM_N = 512              # moving-operand free dim per matmul
WARM_N = 512            # dummy-matmul width

F32 = mybir.dt.float32
BF16 = mybir.dt.bfloat16
NP_BF16 = ml_dtypes.bfloat16

GATHER_SLOTS = 16       # slot 0: local colsum(lin); slots 8+sender: remote

# input DMA chunks: large early (DMA efficiency), tapered tail so the
# final colsum-carrying drain starts as soon as possible
IN_WIDTHS = [4096, 4096, 4096, 4096, 4096, 2048, 1024, 1024, 512]
assert sum(IN_WIDTHS) == R
# chunk index -> issued on Scalar ring (it ramps slowly; Sync carries the
# early chunks so the PE can start on chunk 0 right after warmup)
SCALAR_IN = {3, 5, 7}

# compute tiles (PSUM granularity): 24 x 1024 + 512 tail
TILE_WIDTHS = [PS_N] * 24 + [512]
assert sum(TILE_WIDTHS) == R

# drain engine per tile: DVE(0.96GHz) odd + tail, ACT(1.2GHz) even.
# Tail on DVE so the local-colsum combine (also DVE) follows directly.
DVE_TILES = set(range(1, 24, 2)) | {24}

OUT_WIDTHS = [2048, 4096, 4096, 4096, 4096, 4096, 2048, 512]
assert sum(OUT_WIDTHS) == R

N_WARM_MM = 10          # ~4.3us of dummy matmuls bridges to chunk-0 arrival


def _offsets(widths):
    out, c0 = [], 0
    for w in widths:
        out.append((c0, w))
        c0 += w
    return out


def _move_before(nc, inst, target):
    """Move a post-TileContext instruction directly before `target` in the
    block that holds it (engine dispatch follows list order per engine)."""
    src = dst = None
    for bb in nc.m.functions[0].blocks:
        names = [i.name for i in bb.instructions]
        if inst.name in names:
            src = bb
        if target.name in names:
            dst = bb
    assert src is not None and dst is not None
    src.instructions.remove(inst)
    dst.instructions.insert(dst.instructions.index(target), inst)


def _move_after(nc, inst, target):
    src = dst = None
    for bb in nc.m.functions[0].blocks:
        names = [i.name for i in bb.instructions]
        if inst.name in names:
            src = bb
        if target.name in names:
            dst = bb
    assert src is not None and dst is not None
    src.instructions.remove(inst)
    dst.instructions.insert(dst.instructions.index(target) + 1, inst)


def build_nc(r: int):
    in_chunks = _offsets(IN_WIDTHS)
    tiles = _offsets(TILE_WIDTHS)
    out_chunks = _offsets(OUT_WIDTHS)

    nc = bacc.Bacc(
        "TRN2",
        target_bir_lowering=False,
        debug=False,
        num_devices=N_CORES,
    )

    xt = nc.declare_dram_parameter("xt", [D, r], BF16, isOutput=False)
    w1 = nc.declare_dram_parameter("w1", [D, D], BF16, isOutput=False)
    # wpack: cols 0..127 = w2x = solve(w1, w2)/n, col 128 = bias (one DMA,
    # >=512B per-partition descriptors; separate tiny DMAs clog the ring)
    wpack = nc.declare_dram_parameter("wpack", [D, D + 1], F32, isOutput=False)
    out = nc.declare_dram_parameter("out", [D, r], BF16, isOutput=True)

    # Dummy collective for rank-coordinated launch; nothing waits on it.
    ccw_in = nc.dram_tensor("ccw_in", [D, 1], F32)
    ccw_out = nc.dram_tensor("ccw_out", [D, 1], F32, addr_space="Shared")
    warm_sem = nc.alloc_semaphore("warm_cc")
    nc.gpsimd.collective_compute(
        "AllReduce",
        mybir.AluOpType.add,
        replica_groups=[list(range(N_CORES))],
        ins=[ccw_in.ap().opt()],
        outs=[ccw_out.ap().opt()],
    ).then_inc(warm_sem)

    gsem = nc.alloc_semaphore("gather_sem")
    lsem = nc.alloc_semaphore("rdma_local")
    cs_sem = nc.alloc_semaphore("cs_done")

    # Fixed-address gather buffer (remote cores write slots 8..15).
    gather_sb = nc.alloc_sbuf_tensor("gather_sb", [D, GATHER_SLOTS], F32)

    n_tiles = len(tiles)

    with tile.TileContext(nc) as tc:
        with (
            tc.tile_pool(name="const", bufs=1) as const_pool,
            tc.tile_pool(name="xres", bufs=1) as xres_pool,
            tc.tile_pool(name="obuf", bufs=1) as obuf_pool,
            tc.tile_pool(name="mm", bufs=2, space=bass.MemorySpace.PSUM) as mm_pool,
        ):
            w1_sb = const_pool.tile([D, D], BF16)
            wpack_sb = const_pool.tile([D, D + 1], F32)
            w2x_sb = wpack_sb[:, 0:D]
            bias_sb = wpack_sb[:, D : D + 1]
            zeros_sb = const_pool.tile([D, WARM_N], BF16)
            cs_parts = const_pool.tile([D, n_tiles], F32)
            t_sb = const_pool.tile([D, 1], F32)

            nc.gpsimd.memset(gather_sb[:, :], 0.0)
            nc.gpsimd.memset(zeros_sb[:, :], 0.0)

            # broadcast destination slot offset: 8 + my core id (elements)
            off_gp = nc.gpsimd.alloc_register("slot_off")
            nc.gpsimd.reg_load(off_gp, nc.partition_id_tensor[0:1, 0:1])
            nc.gpsimd.reg_add(off_gp, off_gp, 8)

            # one broadcast: my slot 0 -> peers' slot 8+my_id (self = None)
            slot_out = bass.AP(gather_sb, off_gp, [[GATHER_SLOTS, D], [1, 1]])
            rdests = [None] + [(0, k) for k in range(1, N_CORES)]
            nc.gpsimd.remote_dma_broadcast(
                slot_out,
                gather_sb[:, 0:1],
                gsem,
                lsem,
                rdests=rdests,
            )

            # PE HAM warmup: ~4.3us of dummy matmuls on zeros (no deps)
            warm_ps = mm_pool.tile([D, WARM_N], F32, tag="ps")
            for _ in range(N_WARM_MM):
                nc.tensor.matmul(warm_ps[:, :], zeros_sb[:, 0:D], zeros_sb[:, :])

            # w1 first on the (fast-ramping) Sync ring, then chunk 0
            nc.sync.dma_start(w1_sb[:], w1[:, :])

            x_sb = xres_pool.tile([D, r], BF16)
            for c, (c0, cw) in enumerate(in_chunks):
                eng = nc.scalar if c in SCALAR_IN else nc.sync
                eng.dma_start(x_sb[:, c0 : c0 + cw], xt[:, c0 : c0 + cw])

            nc.scalar.dma_start(wpack_sb[:], wpack[:, :])

            # matmul + drain(+colsum accum) per tile, inside the DMA shadow
            ob = obuf_pool.tile([D, r], BF16)
            for i, (c0, cw) in enumerate(tiles):
                ps = mm_pool.tile([D, PS_N], F32, tag="ps")
                s0 = 0
                while s0 < cw:
                    sw = min(MM_N, cw - s0)
                    nc.tensor.matmul(
                        ps[:, s0 : s0 + sw],
                        w1_sb[:],
                        x_sb[:, c0 + s0 : c0 + s0 + sw],
                    )
                    s0 += sw
                if i in DVE_TILES:
                    # out = in0 + 0; accum_out = reduce(out, op1=add)
                    nc.vector.tensor_scalar(
                        out=ob[:, c0 : c0 + cw],
                        in0=ps[:, :cw],
                        scalar1=0.0,
                        scalar2=None,
                        op0=mybir.AluOpType.add,
                        op1=mybir.AluOpType.add,
                        accum_out=cs_parts[:, i : i + 1],
                    )
                else:
                    nc.scalar.activation(
                        ob[:, c0 : c0 + cw],
                        ps[:, :cw],
                        mybir.ActivationFunctionType.Copy,
                        accum_out=cs_parts[:, i : i + 1],
                    )

            # local colsum(lin) -> gather slot 0, gate + fire the exchange
            cs_reduce = nc.vector.reduce_sum(
                gather_sb[:, 0:1], cs_parts[:], axis=mybir.AxisListType.X
            )
            trig = nc.gpsimd.trigger_dma(
                count=None, signals_writable=[gather_sb[:, :]]
            )

            # global colsum(lin) -> t = SL @ w2x + bias
            slg = const_pool.tile([D, 1], F32)
            gcs_reduce = nc.vector.reduce_sum(
                slg[:], gather_sb[:, :], axis=mybir.AxisListType.X
            )
            t_ps = mm_pool.tile([D, PS_N], F32, tag="ps")
            nc.tensor.matmul(t_ps[:, :1], w2x_sb[:], slg[:])
            nc.vector.tensor_scalar(
                out=t_sb[:],
                in0=t_ps[:, :1],
                scalar1=bias_sb[:],
                scalar2=None,
                op0=mybir.AluOpType.add,
            )

            # phase 3: in-place ob += t on DVE (bf16 SBUF 4x), stream out
            for c, (c0, cw) in enumerate(out_chunks):
                nc.vector.tensor_scalar(
                    out=ob[:, c0 : c0 + cw],
                    in0=ob[:, c0 : c0 + cw],
                    scalar1=t_sb[:],
                    scalar2=None,
                    op0=mybir.AluOpType.add,
                )
                (nc.sync if c % 2 == 0 else nc.scalar).dma_start(
                    out[:, c0 : c0 + cw], ob[:, c0 : c0 + cw]
                )

    # Protocol signal + waits, invisible to Tile's scheduling sim:
    #  - cs_sem inc right after the colsum reduce on DVE
    #  - trigger must not fire before the local colsum is written
    #  - the gather reduce must not read before all 7 remote slots landed
    inc_cs = nc.vector.sem_inc(cs_sem, 1)
    _move_after(nc, inc_cs.ins, cs_reduce.ins)
    w_cs = nc.gpsimd.wait_ge(cs_sem, 1)
    _move_before(nc, w_cs.ins, trig.ins)
    w_arr = nc.vector.wait_ge(gsem, 14)
    _move_before(nc, w_arr.ins, gcs_reduce.ins)

    nc.compile()
    return nc


_nc_cache: dict = {}


def _get_nc(r: int):
    if r not in _nc_cache:
        _nc_cache[r] = build_nc(r)
    return _nc_cache[r]


LAST_RESULTS = None


def _execute(x, w1, w2, bias, r, trace=False, tmpdir=None, trace_cores=None):
    global LAST_RESULTS
    x = np.ascontiguousarray(np.asarray(x, dtype=np.float32))
    w1 = np.ascontiguousarray(np.asarray(w1, dtype=np.float32))
    w2 = np.ascontiguousarray(np.asarray(w2, dtype=np.float32))
    bias = np.asarray(bias, dtype=np.float32)
    n, d = x.shape
    assert d == D and r * N_CORES >= n

    xp = np.zeros((N_CORES * r, d), dtype=np.float32)
    xp[:n] = x
    # (8, r, d) -> (8, d, r) pre-transposed bf16 shards
    xts = np.ascontiguousarray(
        xp.reshape(N_CORES, r, d).transpose(0, 2, 1)
    ).astype(NP_BF16)
    w1_bf = w1.astype(NP_BF16)
    # colsum(lin) = colsum(x) @ w1  =>  fold w1^{-1} and 1/n into w2
    w2x = (
        np.linalg.solve(w1.astype(np.float64), w2.astype(np.float64)) / float(n)
    ).astype(np.float32)
    wpack = np.ascontiguousarray(
        np.concatenate([w2x, bias.reshape(1, d).T.astype(np.float32)], axis=1)
    )

    in_maps = [
        {"xt": xts[i], "w1": w1_bf, "wpack": wpack}
        for i in range(N_CORES)
    ]

    nc = _get_nc(r)
    kwargs = {}
    if trace:
        kwargs.update(trace=True, tmpdir=tmpdir)
        if trace_cores is not None:
            kwargs.update(trace_cores=trace_cores)
    res = run_bass_kernel_spmd(nc, in_maps, core_ids=list(range(N_CORES)), **kwargs)
    LAST_RESULTS = res

    yts = [res.results[i]["out"] for i in range(N_CORES)]  # each (D, r) bf16
    y = np.concatenate([yt.T for yt in yts], axis=0)[:n].astype(np.float32)
    return np.ascontiguousarray(y)


def kernel(x, w1, w2, bias):
    return _execute(x, w1, w2, bias, R)
